# revision 19
# baseline (speedup 1.0000x reference)
"""Causal self-attention (B=4, T=2048, C=1024, H=16) on 8 trn2 NeuronCores.

Sharding: tensor-parallel over heads — each core owns 2 heads (128 of the
1024 channel dims). Each core computes its Q/K/V slices from the full x,
runs causal attention for its heads over all batches, and produces a
partial output projection; the host sums the 8 partials (the all-reduce).

Layout trick: attention scores are computed transposed (S^T[tk, tq]) so
softmax needs no on-chip transposes anywhere in the inner loop:
  - S^T = kT.T @ qT              (kT/qT are [head_dim, tokens] in SBUF)
  - P = exp(S^T)                 (no row-max: scores ~ N(0,1), exp is safe)
  - out[tq, d] = P.T @ v_aug     (v_aug has a ones column -> denominator)
  - normalize with a per-partition scalar multiply (tq is the partition dim)
Causality = skip fully-invalid blocks + one 128x128 triangle mask multiply
on the diagonal block.

Schedule: the QKV projection work for token-chunk i+1 is spliced into the
attention j-loop of chunk i so the TensorE keeps busy while ScalarE works
through the exp() backlog (exp is the second-largest engine load).
"""

import sys

if "/opt/trn_rl_repo" not in sys.path:
    sys.path.insert(0, "/opt/trn_rl_repo")

import ml_dtypes
import numpy as np

B, T, C, H = 4, 2048, 1024, 16
HD = C // H          # 64
NCORES = 8
HPC = H // NCORES    # heads per core = 2
DPC = HPC * HD       # channel dims per core = 128
N = B * T            # 8192 tokens
P = 128              # partitions
TCH = 512            # token chunk (psum bank width in fp32)
KB = C // P          # contraction blocks in stage 1 = 8
NTCH = N // TCH      # 16 token chunks overall
NQC = T // TCH       # tq chunks per batch = 4
NTB = T // P         # 128-token blocks per batch = 16

BF16 = ml_dtypes.bfloat16
REPEAT = 1           # >1 wraps the body in a hardware loop (for benchmarking)
ABLATE = set()       # dev-only: {"exp","av","st","proj","dmaout","s1"}

_CACHE = {}


def _build_nc(repeat=None):
    import concourse.tile as tile
    from concourse import bacc, mybir

    repeat = REPEAT if repeat is None else repeat
    nc = bacc.Bacc(None, target_bir_lowering=False)
    f32 = mybir.dt.float32
    bf16 = mybir.dt.bfloat16
    AF = mybir.ActivationFunctionType

    # ---- DRAM I/O (per-core tensors; same program on all 8 cores) ----
    xt_d = nc.dram_tensor("xt", [C, N], bf16, kind="ExternalInput")
    wq_d = nc.dram_tensor("wq", [C, DPC], bf16, kind="ExternalInput")
    wk_d = nc.dram_tensor("wk", [C, DPC], bf16, kind="ExternalInput")
    wv_d = nc.dram_tensor("wv", [C, DPC], bf16, kind="ExternalInput")
    wp_d = nc.dram_tensor("wp", [DPC, C], bf16, kind="ExternalInput")
    bq_d = nc.dram_tensor("bq", [DPC, 1], f32, kind="ExternalInput")
    bk_d = nc.dram_tensor("bk", [DPC, 1], f32, kind="ExternalInput")
    tri_d = nc.dram_tensor("tri", [P, P], bf16, kind="ExternalInput")
    id_d = nc.dram_tensor("idn", [P, P], bf16, kind="ExternalInput")
    out_d = nc.dram_tensor("out", [N, C], bf16, kind="ExternalOutput")

    with tile.TileContext(nc) as tc:
        with (
            tc.tile_pool(name="persist", bufs=1) as persist,
            tc.tile_pool(name="xp", bufs=4) as xp,
            tc.tile_pool(name="xq", bufs=4) as xq,
            tc.tile_pool(name="ptp", bufs=12) as ptp,
            tc.tile_pool(name="ysp", bufs=10) as ysp,
            tc.tile_pool(name="ytp", bufs=4) as ytp,
            tc.tile_pool(name="osp", bufs=2) as osp,
            tc.tile_pool(name="rcp", bufs=4) as rcp,
            tc.tile_pool(name="s1p", bufs=2, space="PSUM") as s1p,
            tc.tile_pool(name="stp", bufs=2, space="PSUM") as stp,
            tc.tile_pool(name="big", bufs=2, space="PSUM") as big,
            tc.tile_pool(name="avp", bufs=2, space="PSUM") as avp,
        ):
            # ---- persistent SBUF ----
            qTs = persist.tile([P, N], bf16, tag="qTs")   # [dims, tokens]
            kTs = persist.tile([P, N], bf16, tag="kTs")
            # v blocks: per 128-token block: [v_h0 | 1 | v_h1 | 1] = 130 cols
            vs = persist.tile([P, (N // P) * 130], bf16, tag="vs")
            wqs = persist.tile([P, C], bf16, tag="wqs")   # 8 blocks of [128,128]
            wks = persist.tile([P, C], bf16, tag="wks")
            wvs = persist.tile([P, C], bf16, tag="wvs")
            wps = persist.tile([P, C], bf16, tag="wps")
            bqs = persist.tile([P, 1], f32, tag="bqs")
            bks = persist.tile([P, 1], f32, tag="bks")
            tri = persist.tile([P, P], bf16, tag="tri")
            idn = persist.tile([P, P], bf16, tag="idn")

            for k in range(KB):
                nc.sync.dma_start(out=wqs[:, k * P:(k + 1) * P],
                                  in_=wq_d[k * P:(k + 1) * P, :])
                nc.sync.dma_start(out=wks[:, k * P:(k + 1) * P],
                                  in_=wk_d[k * P:(k + 1) * P, :])
                nc.sync.dma_start(out=wvs[:, k * P:(k + 1) * P],
                                  in_=wv_d[k * P:(k + 1) * P, :])
            nc.sync.dma_start(out=wps[:, :], in_=wp_d[:, :])
            nc.sync.dma_start(out=bqs[:, :], in_=bq_d[:, :])
            nc.sync.dma_start(out=bks[:, :], in_=bk_d[:, :])
            nc.sync.dma_start(out=tri[:, :], in_=tri_d[:, :])
            nc.sync.dma_start(out=idn[:, :], in_=id_d[:, :])

            vs_r = vs.rearrange("p (t c) -> p t c", c=130)
            vs_r2 = vs.rearrange("p (t h c) -> p t h c", h=HPC, c=HD + 1)
            nc.vector.memset(vs_r[:, :, HD:HD + 1], 1.0)
            nc.vector.memset(vs_r[:, :, 2 * HD + 1:2 * HD + 2], 1.0)

            # ---------------- body ----------------
            all_xts = {}
            loaded = set()
            nset = [0]

            XW = KB * TCH  # one x chunk holds all 8 contraction blocks
            xt_r = xt_d.rearrange("(k p) n -> p k n", p=P)

            def make_xts01(tag):
                # double-buffered chunk-0/1 x tiles: one body copy's tail
                # load fills the set the next body copy reads (ping-pong)
                return {t: xq.tile([P, XW], bf16, tag="x01",
                                   name=f"x01{tag}_{t}")
                        for t in (0, 1)}

            def emit_loads01(dst):
                for t in (0, 1):
                    t0 = t * TCH
                    nc.sync.dma_start(out=dst[t][:],
                                      in_=xt_r[:, :, t0:t0 + TCH])

            def alloc_xts(tch):
                if tch >= NTCH or tch in all_xts:
                    return
                all_xts[tch] = xp.tile([P, XW], bf16, tag="xt",
                                       name=f"xt{tch}")

            def load_chunk(tch):
                if tch >= NTCH or tch in loaded or tch < 2:
                    return
                loaded.add(tch)
                alloc_xts(tch)
                t0 = tch * TCH
                nc.sync.dma_start(out=all_xts[tch][:],
                                  in_=xt_r[:, :, t0:t0 + TCH])

            def s1_units(tch):
                """QKV projection for token chunk `tch`, as a list of work
                units (callables) to be spliced between attention steps."""
                if tch >= NTCH:
                    return []
                t0 = tch * TCH
                state = {}

                def qk_mms(k):
                    def f():
                        if k == 0:
                            state["psq"] = s1p.tile([P, TCH], f32, tag="s1",
                                                    name=f"psq{tch}")
                            state["psk"] = s1p.tile([P, TCH], f32, tag="s1",
                                                    name=f"psk{tch}")
                        xts = all_xts[tch]
                        xk = xts[:, k * TCH:(k + 1) * TCH]
                        nc.tensor.matmul(state["psq"][:],
                                         wqs[:, k * P:(k + 1) * P], xk,
                                         start=(k == 0), stop=(k == KB - 1))
                        nc.tensor.matmul(state["psk"][:],
                                         wks[:, k * P:(k + 1) * P], xk,
                                         start=(k == 0), stop=(k == KB - 1))
                        if k == KB - 1:
                            nc.vector.tensor_scalar_add(qTs[:, t0:t0 + TCH],
                                                        state["psq"][:],
                                                        bqs[:, :])
                            nc.vector.tensor_scalar_add(kTs[:, t0:t0 + TCH],
                                                        state["psk"][:],
                                                        bks[:, :])
                    return f

                def v_mms(m):
                    def f():
                        xts = all_xts[tch]
                        tb = (t0 + m * P) // P
                        psv = big.tile([P, P], f32, tag="big", name=f"psv{tch}_{m}")
                        for k in range(KB):
                            nc.tensor.matmul(psv[:],
                                             xts[:, k * TCH + m * P:
                                                 k * TCH + (m + 1) * P],
                                             wvs[:, k * P:(k + 1) * P],
                                             start=(k == 0), stop=(k == KB - 1))
                        psv_r = psv.rearrange("p (h c) -> p h c", c=HD)
                        nc.vector.tensor_copy(vs_r2[:, tb, :, 0:HD],
                                              psv_r[:, :, :])
                    return f

                units = []
                for k in range(KB):
                    units.append(qk_mms(k))
                for m in range(TCH // P):
                    units.append(v_mms(m))
                return units

            def att_chunk(b, c, splice):
                """Attention for tq chunk c of batch b, with `splice` work
                units interleaved into the j loop."""
                base = b * T
                q0 = base + c * TCH
                J = 4 * c + 4
                ys_tiles = []
                for m in range(NQC):
                    yt_ = ysp.tile([P, P], bf16, tag="ys", name=f"ys{b}_{c}_{m}")
                    ys_tiles.append(yt_)
                avpair = [avp.tile([P, 4 * (HD + 1)], f32, tag="av",
                                   name=f"avpair_{b}_{c}_{p}")
                          for p in range(NQC // 2)]
                av2 = [avpair[m // 2][:, (m % 2) * 2 * (HD + 1):
                                      (m % 2 + 1) * 2 * (HD + 1)]
                       for m in range(NQC)]
                pts = {}
                rcs = {}
                si = 0
                nsplice = len(splice)

                def do_splice(upto):
                    nonlocal si
                    while si < min(upto, nsplice):
                        splice[si]()
                        si += 1

                LAG = 4
                for j in range(J + LAG):
                    avq = []
                    if j >= LAG:
                        jj = j - LAG
                        for h in range(HPC):
                            pt = pts.pop((jj, h))
                            for m in range(NQC):
                                if jj > 4 * c + m:
                                    continue
                                vtb = b * NTB + jj
                                nn = 1 if "av" in ABLATE else HD + 1

                                def av_unit(pt=pt, h=h, m=m, jj=jj, vtb=vtb,
                                            nn=nn):
                                    nc.tensor.matmul(
                                        av2[m][:, h * (HD + 1):
                                               h * (HD + 1) + nn],
                                        pt[:, m * P:(m + 1) * P],
                                        vs_r[:, vtb,
                                             h * (HD + 1):h * (HD + 1) + nn],
                                        start=(jj == 0 and h == 0
                                               and m % 2 == 0),
                                        stop=(h == 1 and m % 2 == 1
                                              and jj == 4 * c + m))
                                avq.append(av_unit)

                    def pop_av(k):
                        for _ in range(k):
                            if avq:
                                avq.pop(0)()

                    if j < J:
                        r = j - 4 * c
                        u0 = max(0, r) * P
                        for h in range(HPC):
                            hq = h * HD
                            st = stp.tile([P, TCH], f32, tag="st",
                                          name=f"st{b}_{c}_{j}_{h}")
                            if "st" in ABLATE:
                                nc.tensor.matmul(
                                    st[:, u0:u0 + 1],
                                    kTs[hq:hq + HD, base + j * P:base + (j + 1) * P],
                                    qTs[hq:hq + HD, q0 + u0:q0 + u0 + 1],
                                    start=True, stop=True)
                            else:
                                nc.tensor.matmul(
                                    st[:, u0:TCH],
                                    kTs[hq:hq + HD, base + j * P:base + (j + 1) * P],
                                    qTs[hq:hq + HD, q0 + u0:q0 + TCH],
                                    start=True, stop=True)
                            pt = ptp.tile([P, TCH], bf16, tag="pt",
                                          name=f"pt{b}_{c}_{j}_{h}")
                            if "exp" in ABLATE:
                                nc.scalar.activation(pt[:, u0:u0 + 1],
                                                     st[:, u0:u0 + 1], AF.Exp)
                            else:
                                nc.scalar.activation(pt[:, u0:TCH], st[:, u0:TCH],
                                                     AF.Exp)
                            if r >= 0:
                                nc.gpsimd.tensor_mul(pt[:, u0:u0 + P],
                                                     pt[:, u0:u0 + P], tri[:])
                            pts[(j, h)] = pt
                            pop_av(2)
                    # splice stage-1 work for the next chunk between steps,
                    # alternating with the remaining A@V matmuls
                    target = (j + 1) * nsplice // (J + LAG)
                    while avq or si < target:
                        if si < target:
                            splice[si]()
                            si += 1
                        pop_av(2)
                    if j >= LAG:
                        jj = j - LAG
                        # normalize a pair as soon as its accumulation group
                        # closed (group spans both m of the pair), so the psum
                        # bank frees before the chunk tail
                        for m in range(NQC):
                            if jj != 4 * c + (m | 1):
                                continue
                            if m % 2 == 0:
                                # batched reciprocal: all 4 denominators of
                                # this m-pair (2 m x 2 h) in one op, read
                                # strided straight out of the psum tile
                                p_ = m // 2
                                av_s = avpair[p_].rearrange(
                                    "p (k c) -> p k c", c=HD + 1)
                                rcs[p_] = rcp.tile([P, 4], f32, tag="rec",
                                                   name=f"rc{b}_{c}_{p_}")
                                nc.vector.reciprocal(
                                    rcs[p_][:], av_s[:, :, HD:HD + 1])
                            rc = rcs[m // 2]
                            for h in range(HPC):
                                o = h * (HD + 1)
                                k_ = 2 * (m % 2) + h
                                if b == B - 1 and c == NQC - 1:
                                    # last chunk: normalize on the (idle)
                                    # Activation engine so the avp PSUM
                                    # slot frees before the copy boundary
                                    # instead of queuing behind the proj
                                    # osb copies on DVE
                                    nc.scalar.activation(
                                        ys_tiles[m][:, h * HD:(h + 1) * HD],
                                        av2[m][:, o:o + HD],
                                        AF.Identity,
                                        scale=rc[:, k_:k_ + 1])
                                else:
                                    nc.vector.tensor_scalar_mul(
                                        ys_tiles[m][:, h * HD:(h + 1) * HD],
                                        av2[m][:, o:o + HD],
                                        rc[:, k_:k_ + 1])
                do_splice(nsplice)
                return ys_tiles

            out_r = out_d.rearrange("(t p) c -> p t c", p=P)

            def proj_units(b, c, ys_tiles):
                if "proj" in ABLATE:
                    return []
                state_c = {}
                units = []
                for m in range(NQC):
                    state = {}

                    def tr_unit(m=m, state=state):
                        tr_ps = big.tile([P, P], bf16, tag="big",
                                         name=f"tr{b}_{c}_{m}")
                        nc.tensor.transpose(tr_ps[:], ys_tiles[m][:], idn[:])
                        yt = ytp.tile([P, P], bf16, tag="yt",
                                      name=f"yt{b}_{c}_{m}")
                        nc.vector.tensor_copy(yt[:], tr_ps[:])
                        state["yt"] = yt
                        if m == 0:
                            state_c["osb"] = osp.tile([P, NQC * C], bf16,
                                                      tag="os",
                                                      name=f"os{b}_{c}")

                    def mm_unit(oc, m=m, state=state):
                        def f():
                            pp = big.tile([P, TCH], f32, tag="big",
                                          name=f"pp{b}_{c}_{m}_{oc}")
                            nc.tensor.matmul(pp[:], state["yt"][:],
                                             wps[:, oc * TCH:(oc + 1) * TCH],
                                             start=True, stop=True)
                            nc.vector.tensor_copy(
                                state_c["osb"][:, m * C + oc * TCH:
                                               m * C + (oc + 1) * TCH],
                                pp[:])
                            if m == NQC - 1 and oc == 1:
                                tb0 = b * NTB + c * NQC
                                if "dmaout" not in ABLATE:
                                    nc.sync.dma_start(
                                        out=out_r[:, tb0:tb0 + NQC, :],
                                        in_=state_c["osb"][:])
                        return f

                    units += [tr_unit, mm_unit(0), mm_unit(1)]
                return units

            def body(cur01, next01, emit_tail=True):
                all_xts.clear()
                loaded.clear()
                all_xts[0] = cur01[0]
                all_xts[1] = cur01[1]
                for u in s1_units(0):
                    u()
                pending_proj = []
                for i in range(NTCH):
                    b, c = divmod(i, NQC)
                    splice = [lambda i=i: load_chunk(i + 2)]
                    if i == 8 and emit_tail:
                        # prefetch the NEXT body copy's chunk-0/1 x tiles
                        # mid-body: transfers finish long before the loop
                        # barrier, so the drain doesn't wait on them
                        splice.append(lambda: emit_loads01(next01))
                    if i >= NTCH - 2:
                        # final chunks: s1 BEFORE proj so chunk-15's bias
                        # adds free the s1p psum slot well before the copy
                        # boundary (they otherwise gate the next copy's
                        # stage-1 restart through the pool rotation)
                        splice += s1_units(i + 1) + pending_proj
                    else:
                        splice += pending_proj + s1_units(i + 1)
                    ys_tiles = att_chunk(b, c, splice)
                    pending_proj = proj_units(b, c, ys_tiles)
                for u in pending_proj:
                    u()

            seta = make_xts01("a")
            if repeat > 1:
                from concourse import mybir as _mb
                setb = make_xts01("b")
                emit_loads01(seta)
                UNROLL = 3
                nloop, rem = divmod(repeat, UNROLL)
                sets = [seta, setb]
                if nloop > 0:
                    with tc.For_i(0, nloop, 1, hint_engines=(
                            _mb.EngineType.PE, _mb.EngineType.Activation,
                            _mb.EngineType.DVE, _mb.EngineType.SP,
                            _mb.EngineType.Pool)):
                        for u in range(UNROLL):
                            body(sets[u % 2], sets[(u + 1) % 2])
                for r in range(rem):
                    body(sets[r % 2], sets[(r + 1) % 2],
                         emit_tail=(r < rem - 1))
            else:
                emit_loads01(seta)
                body(seta, seta, emit_tail=False)
    nc.compile()
    return nc


def _get_nc():
    if "nc" not in _CACHE:
        _CACHE["nc"] = _build_nc()
    return _CACHE["nc"]


def _make_in_maps(x, Wk, bk, Wq, bq, Wv, bv, Wp, bp):
    x2 = np.ascontiguousarray(np.asarray(x, np.float32).reshape(N, C).T)
    xt = x2.astype(BF16)
    scale = 1.0 / np.sqrt(HD)
    wqt = (np.asarray(Wq, np.float32).T * scale).astype(BF16)
    wkt = np.asarray(Wk, np.float32).T.astype(BF16)
    wvt = np.asarray(Wv, np.float32).T.astype(BF16)
    wpt = np.asarray(Wp, np.float32).T.astype(BF16)
    tri = np.triu(np.ones((P, P), np.float32)).astype(BF16)
    idn = np.eye(P, dtype=np.float32).astype(BF16)
    in_maps = []
    for cidx in range(NCORES):
        s = slice(cidx * DPC, (cidx + 1) * DPC)
        in_maps.append({
            "xt": xt,
            "wq": np.ascontiguousarray(wqt[:, s]),
            "wk": np.ascontiguousarray(wkt[:, s]),
            "wv": np.ascontiguousarray(wvt[:, s]),
            "wp": np.ascontiguousarray(wpt[s, :]),
            "bq": (np.asarray(bq, np.float32)[s] * scale).reshape(DPC, 1),
            "bk": np.asarray(bk, np.float32)[s].reshape(DPC, 1),
            "tri": tri,
            "idn": idn,
        })
    return in_maps


def kernel(x, Wk, bk, Wq, bq, Wv, bv, Wp, bp):
    from concourse.bass_utils import run_bass_kernel_spmd

    nc = _get_nc()
    in_maps = _make_in_maps(x, Wk, bk, Wq, bq, Wv, bv, Wp, bp)
    res = run_bass_kernel_spmd(nc, in_maps, core_ids=list(range(NCORES)))
    acc = np.zeros((N, C), np.float64)
    for r in res.results:
        acc += r["out"].astype(np.float64)
    # v-bias is not applied on-device: attention weights sum to 1, so its
    # contribution to the output is the constant row bv @ Wp.T
    corr = np.asarray(bp, np.float64) + (
        np.asarray(bv, np.float64) @ np.asarray(Wp, np.float64).T)
    out = (acc + corr).astype(np.float32)
    return out.reshape(B, T, C)

